# revision 18
# baseline (speedup 1.0000x reference)
"""Distributed Trainium2 kernel: LayerNorm -> QKV -> causal MHA -> out-proj.

Sharding (8 cores):
  - LayerNorm + QKV projection + final projection: token-parallel
    (4096 tokens -> 512/core).
  - Attention: head-parallel (16 heads -> 2/core).
  - Comms: AllToAll of qkv^T (bf16, 3MB) after the QKV projection
    (re-shards token-parallel -> head-parallel; 2.6MB wire per core vs
    7MB for an xn AllGather); AllToAll of per-head attention output
    (1MB) before the final projection.

Layout notes:
  - All activations are kept TRANSPOSED ([feature, token]) so every matmul
    contraction runs over the partition axis.  S is computed transposed
    (S^T[j,i] = k_j . q_i); softmax sums come from an appended ones-column
    on V (m=65 matmul); causal masking zeroes exp(S^T) diagonal tiles via
    gpsimd affine_select; the dh^-0.5 scale rides the exp free affine.
  - Matmul inputs are bf16 (4x the fp32 TensorE rate); accumulation fp32.
    Weights arrive host-pre-tiled [p, k, cols] (contiguous DMAs), with
    qkv feature order [rank][q|k|v][128] so the A2A slices are direct.
"""

import numpy as np
import ml_dtypes

import concourse.bass as bass
import concourse.tile as tile
from concourse import bacc, mybir
from concourse.bass import ds, ts
from concourse.bass_utils import run_bass_kernel_spmd
from concourse.masks import make_identity

B, N, D = 2, 2048, 1024
HEADS, DH = 16, 64
INNER = HEADS * DH          # 1024
NCORES = 8
T = B * N                   # 4096 tokens
TS = T // NCORES            # 512 tokens per core
HPC = HEADS // NCORES       # 2 heads per core
SCALE = float(DH) ** -0.5   # 0.125
EPS = 1e-5

FP = mybir.dt.float32
BF = mybir.dt.bfloat16

KT = D // 128               # 8 contraction tiles of 128 over dim
TCH = T // 512              # 8 token chunks of 512 (== ranks)
ICB = N // 512              # 4 i-chunks of 512 per batch
JTB = N // 128              # 16 j-tiles of 128 per batch
FT = 3 * INNER // 128       # 24 qkv feature tiles of 128


def build():
    nc = bacc.Bacc("TRN2", target_bir_lowering=False, debug=False,
                   num_devices=NCORES)

    x_sh = nc.dram_tensor("x_shard", [TS, D], FP, kind="ExternalInput")
    gamma_t = nc.dram_tensor("gamma", [D], FP, kind="ExternalInput")
    beta_t = nc.dram_tensor("beta", [D], FP, kind="ExternalInput")
    # host-pre-tiled: [p, k, f] with f = rank*384 + proj*128 + c
    wqkv_t = nc.dram_tensor("wqkv", [128, KT, 3 * INNER], BF,
                            kind="ExternalInput")
    wo_t = nc.dram_tensor("w_out", [128, KT, D], BF, kind="ExternalInput")
    out_sh = nc.dram_tensor("out_shard", [TS, D], FP, kind="ExternalOutput")

    with tile.TileContext(nc) as tc:
        _body(nc, tc, x_sh, gamma_t, beta_t, wqkv_t, wo_t, out_sh)

    nc.compile()
    return nc


def _att_thunks(nc, b, ic, kTt, qT, vhat, outT,
                s_ps, av_ps, espool, smallp):
    """Attention for query chunk (b, ic) as a list of PE-ordered thunks.

    Pipelined: S-matmuls for step jp are emitted before the AV-matmuls of
    step jp-1, so the PE never sits directly behind the ACT exp latency.
    """
    q_idx = b * ICB + ic
    njt = 4 * (ic + 1)
    av = [av_ps.tile([128, 512], FP, tag=f"av{h}", name=f"av{h}_{q_idx}")
          for h in range(HPC)]
    es = {}

    def spair(h, jp):
        def run():
            sx = s_ps.tile([128, 1024], FP, tag="sx",
                           name=f"sx{h}_{q_idx}_{jp}")
            e = espool.tile([128, 1024], BF, tag="es",
                            name=f"es{h}_{q_idx}_{jp}")
            es[(h, jp)] = e
            for u in range(2):
                jt = 2 * jp + u
                tq = b * ICB + jt // 4
                jo = 128 * (jt % 4)
                nc.tensor.matmul(
                    sx[:, ds(512 * u, 512)],
                    kTt[ds(64 * h, 64), tq, ds(jo, 128)],
                    qT[ds(64 * h, 64), q_idx, :],
                    start=True, stop=True,
                    tile_position=(64 * h, 0))
            nc.scalar.activation(
                out=e, in_=sx,
                func=mybir.ActivationFunctionType.Exp, scale=SCALE)
        return run

    def avpair(h, jp):
        def run():
            e = es.pop((h, jp))
            for u in range(2):
                jt = 2 * jp + u
                m = jt - 4 * ic
                if 0 <= m < 4:
                    # causal: keep e[j, i] iff i - j - 128*m >= 0, else 0
                    nc.gpsimd.affine_select(
                        out=e[:, ds(512 * u, 512)],
                        in_=e[:, ds(512 * u, 512)],
                        compare_op=mybir.AluOpType.is_ge, fill=0.0,
                        base=-128 * m, pattern=[[1, 512]],
                        channel_multiplier=-1)
                nc.tensor.matmul(
                    av[h][0:65, :],
                    vhat[:, b * JTB + jt, ds(65 * h, 65)],
                    e[:, ds(512 * u, 512)],
                    start=(jt == 0), stop=(jt == njt - 1))
        return run

    def norm(h):
        def run():
            rsum = smallp.tile([1, 512], FP, tag="rsum", name=f"rs{h}_{q_idx}")
            nc.vector.tensor_copy(out=rsum, in_=av[h][64:65, :])
            rec = smallp.tile([1, 512], FP, tag="rec", name=f"rc{h}_{q_idx}")
            nc.vector.reciprocal_approx_fast(out=rec, in_=rsum)
            bc = smallp.tile([64, 512], FP, tag="bc", name=f"bc{h}_{q_idx}")
            nc.gpsimd.partition_broadcast(bc, rec)
            nc.vector.tensor_tensor(
                out=outT[h][:, ds(512 * q_idx, 512)],
                in0=av[h][0:64, :], in1=bc,
                op=mybir.AluOpType.mult)
        return run

    thunks = []
    nps = njt // 2
    thunks.append(spair(0, 0))
    thunks.append(spair(1, 0))
    for jp in range(1, nps):
        thunks.append(spair(0, jp))
        thunks.append(avpair(0, jp - 1))
        thunks.append(spair(1, jp))
        thunks.append(avpair(1, jp - 1))
    thunks.append(avpair(0, nps - 1))
    thunks.append(avpair(1, nps - 1))
    thunks.append(norm(0))
    thunks.append(norm(1))
    return thunks


def _body(nc, tc, x_sh, gamma_t, beta_t, wqkv_t, wo_t, out_sh):
    from contextlib import ExitStack
    ctx = ExitStack()
    with ctx:
        const = ctx.enter_context(tc.tile_pool(name="const", bufs=1))
        wpool = ctx.enter_context(tc.tile_pool(name="wpool", bufs=1))
        big = ctx.enter_context(tc.tile_pool(name="big", bufs=1))
        dram = ctx.enter_context(tc.tile_pool(name="dram", bufs=1, space="DRAM"))

        # ---------- constants ----------
        identity = const.tile([128, 128], BF)
        make_identity(nc, identity)

        eps_t = const.tile([128, 1], FP)
        nc.vector.memset(eps_t, EPS)

        # gamma/beta broadcast across partitions via PE (ones outer product)
        ones_col = const.tile([1, 128], BF)
        nc.vector.memset(ones_col, 1.0)
        gb_row = const.tile([1, 2, D], FP)
        g_ap = gamma_t.ap()
        b_ap = beta_t.ap()
        nc.sync.dma_start(out=gb_row[:, 0, :], in_=bass.AP(
            tensor=g_ap.tensor, offset=g_ap.offset,
            ap=[[0, 1]] + list(g_ap.ap)))
        nc.sync.dma_start(out=gb_row[:, 1, :], in_=bass.AP(
            tensor=b_ap.tensor, offset=b_ap.offset,
            ap=[[0, 1]] + list(b_ap.ap)))
        gb_bf = const.tile([1, 2, D], BF)
        nc.vector.tensor_copy(out=gb_bf, in_=gb_row)
        gamma_b = const.tile([128, D], BF)
        beta_b = const.tile([128, D], BF)
        with tc.tile_pool(name="bc_ps", bufs=1, space="PSUM") as bc_ps:
            for dst, which in ((gamma_b, 0), (beta_b, 1)):
                bps = bc_ps.tile([128, D], FP, tag="bps")
                for half in range(2):
                    nc.tensor.matmul(
                        bps[:, ds(512 * half, 512)], ones_col,
                        gb_bf[:, which, ds(512 * half, 512)],
                        start=True, stop=True)
                nc.vector.tensor_copy(out=dst, in_=bps)

        # ---------- comm bounce buffers ----------
        qkv_a2a_in = dram.tile([NCORES, 128, 3 * 512], BF)
        qkv_a2a_out = dram.tile([NCORES, 128, 3 * 512], BF)
        a2a_in = dram.tile([NCORES, 128, TS], BF)
        a2a_out = dram.tile([NCORES, 128, TS], BF)

        wqkv_sb = wpool.tile([128, KT, 3 * INNER], BF)
        wo_sb = wpool.tile([128, KT, D], BF)
        xnT_sb = wpool.tile([128, KT, TS], BF)

        # ---------- phase A: LayerNorm + transpose (SBUF-local) ----------
        nc.gpsimd.dma_start(out=wqkv_sb, in_=wqkv_t.ap())
        nc.gpsimd.dma_start(out=wo_sb, in_=wo_t.ap())
        with tc.tile_pool(name="lnp", bufs=3) as lnp, \
             tc.tile_pool(name="lns", bufs=4) as lns:
            x_tiles = []
            for tt in range(TS // 128):
                x_t = lnp.tile([128, D], FP, tag=f"x{tt}", name=f"x_{tt}",
                               bufs=1)
                nc.sync.dma_start(out=x_t, in_=x_sh.ap()[ts(tt, 128), :])
                x_tiles.append(x_t)
            for tt in range(TS // 128):
                x_t = x_tiles[tt]
                stats = lns.tile([128, 2, 6], FP, tag="stats")
                xg = x_t.rearrange("p (s f) -> p s f", f=512)
                for s in range(2):
                    nc.vector.bn_stats(out=stats[:, s, :], in_=xg[:, s, :])
                mv = lns.tile([128, 2], FP, tag="mv")
                nc.vector.bn_aggr(out=mv, in_=stats)
                rstd = lns.tile([128, 1], FP, tag="rstd")
                nc.scalar.activation(out=rstd, in_=mv[:, 1:2],
                                     func=mybir.ActivationFunctionType.Sqrt,
                                     bias=eps_t, scale=1.0)
                nc.vector.reciprocal(out=rstd, in_=rstd)
                xn_f = lnp.tile([128, D], BF, tag="xnf")
                nc.vector.tensor_scalar(
                    out=xn_f, in0=x_t, scalar1=mv[:, 0:1], scalar2=rstd,
                    op0=mybir.AluOpType.subtract, op1=mybir.AluOpType.mult)
                xn_g = lnp.tile([128, D], BF, tag="xng")
                nc.vector.tensor_tensor(out=xn_g, in0=xn_f, in1=gamma_b,
                                        op=mybir.AluOpType.mult)
                xn_bf = lnp.tile([128, D], BF, tag="xnbf")
                nc.vector.tensor_tensor(out=xn_bf, in0=xn_g, in1=beta_b,
                                        op=mybir.AluOpType.add)
                nc.scalar.dma_start_transpose(
                    out=xnT_sb[:, :, ts(tt, 128)], in_=xn_bf)

        # ---------- phase B: QKV^T (token-sharded) + A2A ----------
        with tc.tile_pool(name="qkv_ps", bufs=2, space="PSUM") as qkv_ps, \
             tc.tile_pool(name="qst", bufs=4) as qst:
            for ft in range(FT):
                acc = qkv_ps.tile([128, 512], FP, tag="acc", name=f"qk_{ft}")
                for k in range(KT):
                    nc.tensor.matmul(
                        acc, wqkv_sb[:, k, ds(128 * ft, 128)],
                        xnT_sb[:, k, :],
                        start=(k == 0), stop=(k == KT - 1))
                qf = qst.tile([128, 512], BF, tag="qf", name=f"qf_{ft}")
                nc.vector.tensor_copy(out=qf, in_=acc)
                r, proj = ft // 3, ft % 3
                nc.sync.dma_start(
                    out=qkv_a2a_in[r, :, ds(512 * proj, 512)], in_=qf)

        nc.gpsimd.collective_compute(
            "AllToAll", mybir.AluOpType.bypass,
            replica_groups=[list(range(NCORES))],
            ins=[qkv_a2a_in.opt()], outs=[qkv_a2a_out.opt()])

        # ---------- phase C: receive q/k/v + attention ----------
        qT = big.tile([128, TCH, 512], BF)   # rows: [h0 64 | h1 64]
        kTt = big.tile([128, TCH, 512], BF)
        vhat = big.tile([128, JTB * B, 130], BF)  # [j, jt, 65*h+c]
        nc.gpsimd.memset(vhat[:, :, 64:65], 1.0)
        nc.gpsimd.memset(vhat[:, :, 129:130], 1.0)
        outT = [big.tile([64, T], BF, name=f"outT{h}") for h in range(HPC)]

        with tc.tile_pool(name="s_ps", bufs=2, space="PSUM") as s_ps, \
             tc.tile_pool(name="av_ps", bufs=1, space="PSUM") as av_ps, \
             tc.tile_pool(name="espool", bufs=8) as espool, \
             tc.tile_pool(name="smallp", bufs=4) as smallp, \
             tc.tile_pool(name="vstage", bufs=3) as vst:
            # receive DMAs for every chunk (gated only on the collective)
            for r in range(TCH):
                nc.sync.dma_start(out=qT[:, r, :],
                                  in_=qkv_a2a_out[r][:, 0:512])
                nc.sync.dma_start(out=kTt[:, r, :],
                                  in_=qkv_a2a_out[r][:, 512:1024])
            for r in range(TCH):
                vs = vst.tile([128, 512], BF, tag="vs", name=f"vs_{r}")
                nc.sync.dma_start(out=vs, in_=qkv_a2a_out[r][:, 1024:1536])
                vstg = vst.tile([128, 4, 128], BF, tag="vstg",
                                name=f"vstg_{r}")
                nc.sync.dma_start_transpose(out=vstg, in_=vs)
                nc.vector.tensor_copy(out=vhat[:, ds(r * 4, 4), 0:64],
                                      in_=vstg[:, :, 0:64])
                nc.vector.tensor_copy(out=vhat[:, ds(r * 4, 4), 65:129],
                                      in_=vstg[:, :, 64:128])
            for tci in range(TCH):
                b, ic = tci // ICB, tci % ICB
                for thunk in _att_thunks(nc, b, ic, kTt, qT, vhat,
                                         outT, s_ps, av_ps, espool, smallp):
                    thunk()

        # ---------- phase D: AllToAll + output projection ----------
        for r in range(NCORES):
            nc.sync.dma_start(out=a2a_in[r, 0:64, :],
                              in_=outT[0][:, ds(512 * r, 512)])
            nc.sync.dma_start(out=a2a_in[r, 64:128, :],
                              in_=outT[1][:, ds(512 * r, 512)])
        nc.gpsimd.collective_compute(
            "AllToAll", mybir.AluOpType.bypass,
            replica_groups=[list(range(NCORES))],
            ins=[a2a_in.opt()], outs=[a2a_out.opt()])

        a2a_sb = big.tile([128, NCORES, 512], BF)
        for r in range(NCORES):
            nc.sync.dma_start(out=a2a_sb[:, r, :], in_=a2a_out[r])

        out_view = out_sh.ap().rearrange("(t p) e -> p t e", p=128)
        with tc.tile_pool(name="op_ps", bufs=2, space="PSUM") as op_ps, \
             tc.tile_pool(name="ost", bufs=3) as ostp:
            for tt in range(TS // 128):
                po = [op_ps.tile([128, 512], FP, tag=f"po{ec}",
                                 name=f"po{ec}_{tt}")
                      for ec in range(D // 512)]
                for ct in range(NCORES):
                    for ec in range(D // 512):
                        nc.tensor.matmul(
                            po[ec], a2a_sb[:, ct, ds(128 * tt, 128)],
                            wo_sb[:, ct, ds(512 * ec, 512)],
                            start=(ct == 0), stop=(ct == NCORES - 1))
                for ec in range(D // 512):
                    ost = ostp.tile([128, 512], FP, tag="ost")
                    nc.vector.tensor_copy(out=ost, in_=po[ec])
                    nc.sync.dma_start(out=out_view[:, tt, ds(512 * ec, 512)],
                                      in_=ost)


_NC = None
LAST_EXEC_TIME_NS = None


def _get_nc():
    global _NC
    if _NC is None:
        _NC = build()
    return _NC


def _ptile(w):
    """[D, C] -> [128, KT, C] with p (partition) split out of the rows."""
    c = w.shape[1]
    return np.ascontiguousarray(
        w.reshape(KT, 128, c).transpose(1, 0, 2))


def make_in_maps(x, gamma, beta, w_qkv, w_out):
    bf = ml_dtypes.bfloat16
    x = np.ascontiguousarray(np.asarray(x, dtype=np.float32)).reshape(T, D)
    gamma = np.ascontiguousarray(np.asarray(gamma, dtype=np.float32))
    beta = np.ascontiguousarray(np.asarray(beta, dtype=np.float32))
    w_qkv = np.asarray(w_qkv, dtype=np.float32).astype(bf)
    # reorder qkv features to [rank][q|k|v][128]
    wq = w_qkv[:, :INNER].reshape(D, NCORES, 128)
    wk = w_qkv[:, INNER:2 * INNER].reshape(D, NCORES, 128)
    wv = w_qkv[:, 2 * INNER:].reshape(D, NCORES, 128)
    wqkv = np.stack([wq, wk, wv], axis=2).reshape(D, 3 * INNER)
    wqkv_t = _ptile(wqkv)
    w_out_t = _ptile(np.asarray(w_out, dtype=np.float32).astype(bf))
    in_maps = []
    for c in range(NCORES):
        in_maps.append({
            "x_shard": np.ascontiguousarray(x[TS * c: TS * (c + 1)]),
            "gamma": gamma,
            "beta": beta,
            "wqkv": wqkv_t,
            "w_out": w_out_t,
        })
    return in_maps


def kernel(x, mask, gamma, beta, w_qkv, w_out):
    global LAST_EXEC_TIME_NS
    nc = _get_nc()
    in_maps = make_in_maps(x, gamma, beta, w_qkv, w_out)
    res = run_bass_kernel_spmd(nc, in_maps, core_ids=list(range(NCORES)))
    LAST_EXEC_TIME_NS = res.exec_time_ns
    out = np.concatenate([res.results[c]["out_shard"] for c in range(NCORES)],
                         axis=0)
    return out.reshape(B, N, D).astype(np.float32)
